# revision 13
# baseline (speedup 1.0000x reference)
"""Trainium2 Bass kernel for nn_Attention_26173530702697.

Dense transformer block (sigmoid attention x2, PEG depthwise conv, LN x3,
MLP) on decoder [8, 384, 32, 32]. Sharding: pure data parallel over batch
(B=8 == 8 cores), zero collectives. Everything on a core stays d-major
[384, 1024] (channels on partitions), which makes the PEG conv and all
per-channel affine ops per-partition, and feeds the matmuls directly.

Matmul operands are bf16 (1 cycle/row on the PE) except where fp8 noise
survives the error budget: MHA2's score*value product and both MLP matmuls
run fp8e4 with DoubleRow (2 contraction rows/cycle). fp8 anywhere in MHA1
is amplified ~16x by MHA2's token-sum and blows the 2e-2 budget (measured),
so MHA1 stays bf16 throughout.

The PEG conv is emitted per channel-tile from inside MHA1's last-head
o-projection (center tap on ACT, vertical taps on GpSimd, the rest on the
DVE, merged straight into the bf16 tile LN1 consumes), and each LN's
shadow copies are emitted as soon as their input tile lands -- all so the
PE never idles >3.4us, which would drop the HAM clock gate to half rate
through the PEG/LN1 valley and the MLP tail.
"""

import math
import os

import ml_dtypes
import numpy as np

import concourse.bass as bass
import concourse.tile as tile
from concourse import bacc
from concourse import mybir
from concourse.bass_utils import run_bass_kernel_spmd

F32 = mybir.dt.float32
BF16 = mybir.dt.bfloat16
FP8 = mybir.dt.float8e4
AF = mybir.ActivationFunctionType
OP = mybir.AluOpType
DR = mybir.MatmulPerfMode.DoubleRow

B, DIM, H, W = 8, 384, 32, 32
HEADS, DK = 8, 96
N = H * W            # 1024
C3 = DIM // 128      # 3 channel tiles
H6 = 768 // 128      # 6 hidden tiles
EPS = 1e-5
HALF = 512

LAST_EXEC_TIME_NS = None


def build_nc():
    nc = bacc.Bacc("TRN2", target_bir_lowering=False, debug=False,
                   enable_asserts=True, num_devices=B)

    def _param(name, shape, dt=BF16, out=False):
        return nc.dram_tensor(name, shape, dt,
                              kind="ExternalOutput" if out else "ExternalInput").ap()

    # ---- DRAM parameters (per-core shapes; weights replicated) ----
    x_ext = _param("x", [128, C3, N])
    out_ext = _param("out", [C3, 128, N], BF16, out=True)

    wq_ext, wk_ext, wv_ext = {}, {}, {}
    bv_ext = {}
    for i in (1, 2):
        wq_ext[i] = _param(f"wq{i}", [HEADS, 128, C3, DK])
        wk_ext[i] = _param(f"wk{i}", [HEADS, 128, C3, DK])
        wv_ext[i] = _param(f"wv{i}", [HEADS, 128, C3, DIM])
        bv_ext[i] = _param(f"bv{i}", [HEADS, DIM])
    constf_ext = _param("constf", [128, 96], F32)
    constg_ext = _param("constg", [1, 3 * DIM])
    w1_ext = _param("mlp_w1", [128, C3, 768], FP8)
    w2_ext = _param("mlp_w2", [128, H6, DIM], FP8)

    MM = nc.tensor.matmul

    with tile.TileContext(nc) as tc:
        with (
            tc.tile_pool(name="xp", bufs=12) as xp,
            tc.tile_pool(name="xb", bufs=12) as xb,        # bf16 shadows / LN outs
            tc.tile_pool(name="stat", bufs=5) as stat,
            tc.tile_pool(name="const", bufs=1) as constp,
            tc.tile_pool(name="mlpw", bufs=1) as mlp_p,
        ):
            # ---- input first so its DMA leads the queue ----
            xin = constp.tile([128, C3, N], BF16, name="xin", tag="xin")
            for c in range(C3):
                nc.sync.dma_start(xin[:, c, :], x_ext[:, c, :])

            # ---- constants ----
            ones_row = constp.tile([1, 128], BF16, name="ones_row", tag="ones_row")
            nc.vector.memset(ones_row[:], 1.0)
            inv_col = constp.tile([128, 1], BF16, name="inv_col", tag="inv_col")
            nc.vector.memset(inv_col[:], 1.0 / DIM)
            eps_t = constp.tile([1, 1], F32, name="eps_t", tag="eps_t")
            nc.vector.memset(eps_t[:], EPS)
            # absorb any GpSimd first-dispatch cost before the PEG needs it
            gwu = constp.tile([128, 8], F32, name="gwu", tag="gwu")
            nc.gpsimd.memset(gwu[:], 0.0)

            cf = constp.tile([128, 96], F32, name="cf", tag="cf")
            nc.sync.dma_start(cf[:], constf_ext[:])
            cg = constp.tile([1, 3 * DIM], BF16, name="cg", tag="cg")
            nc.sync.dma_start(cg[:], constg_ext[:])
            # packed fp32 const columns (see _prep_weights)
            bet = {k: cf[:, 3 * j:3 * j + 3]
                   for j, k in enumerate(("ln1", "mlpln", "ln2"))}
            gam = {k: cg[:, j * DIM:(j + 1) * DIM]
                   for j, k in enumerate(("ln1", "mlpln", "ln2"))}
            a_sb = {1: cf[:, 9:12], 2: cf[:, 12:15]}
            a3_sb = cf[:, 15:18]
            bp_sb = {1: cf[:, 18:21], 2: cf[:, 21:24]}
            pegw_sb = cf[:, 24:51].rearrange("p (c t) -> p c t", t=9)
            pegb_sb = cf[:, 51:54]
            b1_sb = cf[:, 54:60]
            b2_sb = cf[:, 60:63]
            bq_sb = {1: cf[0:DK, 63:71], 2: cf[0:DK, 79:87]}
            bk_sb = {1: cf[0:DK, 71:79], 2: cf[0:DK, 87:95]}

            # ---- MLP weights (fp8), prefetched during the MHA phases ----
            w1_sb = mlp_p.tile([128, C3, 768], FP8, name="w1", tag="w1")
            nc.sync.dma_start(w1_sb[:], w1_ext[:])
            w2_sb = mlp_p.tile([128, H6, DIM], FP8, name="w2", tag="w2")
            nc.sync.dma_start(w2_sb[:], w2_ext[:])
            hn_t = mlp_p.tile([128, C3, N], FP8, name="hn", tag="hn")
            hid_t = mlp_p.tile([128, H6, N], FP8, name="hid", tag="hid")

            def layer_norm(x_tiles, key, psum_pool, out_dt=None, out_pool=None,
                           out_aps=None, shadows=None, pre_colsum=None):
                """LN over channel axis (partitions). Colsums with a 1/DIM
                weight column give mu and E[x^2] directly; rsqrt via ACT;
                normalize via rank-1 broadcasts through the PE.

                shadows: optional per-tile (bf16 copy, bf16 square) pairs the
                caller already emitted (so their ACT ops pipeline under
                earlier work instead of serializing here). pre_colsum(c) lets
                the caller emit a PE keep-alive before tile c's column sums.
                """
                g_row, b_col = gam[key], bet[key]
                mu_ps = psum_pool.tile([1, N], F32, name="mu_ps", tag="ps")
                ex2_ps = psum_pool.tile([1, N], F32, name="ex2_ps", tag="ps")
                for c in range(C3):
                    if shadows is not None:
                        xsc, s = shadows[c]
                    else:
                        if x_tiles[c].dtype == BF16:
                            xsc = x_tiles[c]
                        else:
                            xsc = xb.tile([128, N], BF16, name="xs", tag="xb")
                            nc.scalar.copy(xsc[:], x_tiles[c][:])
                        s = xb.tile([128, N], BF16, name="sq", tag="xb")
                        nc.scalar.square(s[:], x_tiles[c][:])
                    if pre_colsum is not None:
                        pre_colsum(c)
                    for hlf in range(2):
                        sl = slice(hlf * HALF, (hlf + 1) * HALF)
                        MM(mu_ps[:, sl], inv_col[:], xsc[:, sl],
                           start=(c == 0), stop=(c == C3 - 1))
                        MM(ex2_ps[:, sl], inv_col[:], s[:, sl],
                           start=(c == 0), stop=(c == C3 - 1))
                mu = stat.tile([1, N], F32, name="mu", tag="stat")
                nc.vector.tensor_copy(mu[:], mu_ps[:])
                mu2 = stat.tile([1, N], F32, name="mu2", tag="stat")
                nc.scalar.square(mu2[:], mu_ps[:])
                var = stat.tile([1, N], F32, name="var", tag="stat")
                nc.vector.scalar_tensor_tensor(
                    var[:], ex2_ps[:], 1.0, mu2[:],
                    op0=OP.mult, op1=OP.subtract)
                rstd = stat.tile([1, N], BF16, name="rstd", tag="stat")
                nc.scalar.activation(rstd[:], var[:], AF.Abs_reciprocal_sqrt,
                                     bias=eps_t[:])
                mc = stat.tile([1, N], BF16, name="mc", tag="stat")
                nc.vector.tensor_mul(mc[:], mu[:], rstd[:])
                A, Cg = [], []
                for c in range(C3):
                    g_seg = g_row[:, c * 128:(c + 1) * 128]
                    Ac = psum_pool.tile([128, N], F32, name="A", tag="ps")
                    for hlf in range(2):
                        sl = slice(hlf * HALF, (hlf + 1) * HALF)
                        MM(Ac[:, sl], g_seg, rstd[:, sl], start=True, stop=True)
                    A.append(Ac)
                for c in range(C3):
                    g_seg = g_row[:, c * 128:(c + 1) * 128]
                    Cc = psum_pool.tile([128, N], F32, name="Cg", tag="ps")
                    for hlf in range(2):
                        sl = slice(hlf * HALF, (hlf + 1) * HALF)
                        MM(Cc[:, sl], g_seg, mc[:, sl], start=True, stop=True)
                    Cg.append(Cc)
                out = []
                for c in range(C3):
                    t1 = xp.tile([128, N], F32, name="t1", tag="x")
                    nc.vector.tensor_mul(t1[:], x_tiles[c][:], A[c][:])
                    if out_aps is not None:
                        y = out_aps[c]
                        nc.vector.scalar_tensor_tensor(
                            y, t1[:], b_col[:, c:c + 1], Cg[c][:],
                            op0=OP.add, op1=OP.subtract)
                        out.append(y)
                    else:
                        y = out_pool.tile([128, N], out_dt, name="lnout",
                                          tag="x" if out_pool is xp else "xb")
                        nc.vector.scalar_tensor_tensor(
                            y[:], t1[:], b_col[:, c:c + 1], Cg[c][:],
                            op0=OP.add, op1=OP.subtract)
                        out.append(y)
                return out

            def mha(i, x_tiles, pools, psum_pool, fp8=False, on_tile_done=None):
                """y = a_i * x + MHA_i(x); x_tiles bf16 d-major; returns fp32.

                Head loop is software-pipelined: head h's O/projector matmuls
                are emitted after head h+1's QKV/score matmuls so the PE
                stream covers the sigmoid latency of head h+1.

                fp8=True stores S^T and V as fp8e4 kc-pair tiles and runs the
                o-projection with DoubleRow (2 contraction rows/cycle).
                on_tile_done(dm) is invoked right after the LAST head's
                o-projection finishes output tile dm, so the caller can emit
                vector work (the PEG conv) that overlaps the remaining tiles.
                """
                wq_p, wv_p, st_p, v_p, qk_p, bvb_p = pools
                Y = []
                for c in range(C3):
                    y = xp.tile([128, N], F32, name="yres", tag="x")
                    nc.vector.tensor_scalar(
                        y[:], x_tiles[c][:], a_sb[i][:, c:c + 1], bp_sb[i][:, c:c + 1],
                        op0=OP.mult, op1=OP.add)
                    Y.append(y)

                def qkvst(h):
                    wq_t = wq_p.tile([128, C3, DK], BF16, name="wq", tag="wq")
                    nc.sync.dma_start(wq_t[:], wq_ext[i][h])
                    wk_t = wq_p.tile([128, C3, DK], BF16, name="wk", tag="wk")
                    nc.sync.dma_start(wk_t[:], wk_ext[i][h])
                    wv_t = wv_p.tile([128, C3, DIM], BF16, name="wv", tag="wv")
                    nc.sync.dma_start(wv_t[:], wv_ext[i][h])
                    bv_row = bvb_p.tile([1, DIM], BF16, name="bvrow", tag="bvrow")
                    nc.sync.dma_start(bv_row[:], bv_ext[i][h].unsqueeze(0))

                    # Q^T, K^T: [96, 1024] d-major (score scale folded into wq)
                    qt_ps = psum_pool.tile([DK, N], F32, name="qt_ps", tag="ps")
                    kt_ps = psum_pool.tile([DK, N], F32, name="kt_ps", tag="ps")
                    qt = qk_p.tile([DK, N], BF16, name="qt", tag="qk")
                    kt = qk_p.tile([DK, N], BF16, name="kt", tag="qk")
                    for c in range(C3):
                        for hlf in range(2):
                            sl = slice(hlf * HALF, (hlf + 1) * HALF)
                            MM(kt_ps[:, sl], wk_t[:, c, :], x_tiles[c][:, sl],
                               start=(c == 0), stop=(c == C3 - 1))
                    for hlf in range(2):
                        sl = slice(hlf * HALF, (hlf + 1) * HALF)
                        nc.vector.tensor_scalar_add(
                            kt[:, sl], kt_ps[:, sl], bk_sb[i][:, h:h + 1])
                    for c in range(C3):
                        for hlf in range(2):
                            sl = slice(hlf * HALF, (hlf + 1) * HALF)
                            MM(qt_ps[:, sl], wq_t[:, c, :], x_tiles[c][:, sl],
                               start=(c == 0), stop=(c == C3 - 1))
                    for hlf in range(2):
                        sl = slice(hlf * HALF, (hlf + 1) * HALF)
                        nc.vector.tensor_scalar_add(
                            qt[:, sl], qt_ps[:, sl], bq_sb[i][:, h:h + 1])

                    bvb_ps = psum_pool.tile([128, DIM], F32, name="bvb_ps", tag="ps")
                    MM(bvb_ps[:], ones_row[:], bv_row[:], start=True, stop=True)
                    bvb = bvb_p.tile([128, DIM], BF16, name="bvb", tag="bvb")
                    nc.vector.tensor_copy(bvb[:], bvb_ps[:])

                    # interleave V and S^T so V matmuls cover sigmoid latency
                    v_sb, st_sb = [], []
                    for kc in range(HEADS):
                        if fp8 and kc % 2 == 0:
                            v_sb.append(v_p.tile([128, 2, DIM], FP8,
                                                 name="v", tag="v"))
                            st_sb.append(st_p.tile([128, 2, N], FP8,
                                                   name="s", tag="st"))
                        ksl = slice(kc * 128, (kc + 1) * 128)
                        v_ps = psum_pool.tile([128, DIM], F32, name="v_ps", tag="ps")
                        for c in range(C3):
                            MM(v_ps[:], x_tiles[c][:, ksl], wv_t[:, c, :],
                               start=(c == 0), stop=(c == C3 - 1))
                        st_ps = psum_pool.tile([128, N], F32, name="st_ps", tag="ps")
                        for hlf in range(2):
                            sl = slice(hlf * HALF, (hlf + 1) * HALF)
                            MM(st_ps[:, sl], kt[:, ksl], qt[:, sl],
                               start=True, stop=True)
                        if fp8:
                            nc.vector.tensor_add(v_sb[-1][:, kc % 2, :],
                                                 v_ps[:], bvb[:])
                            nc.scalar.activation(st_sb[-1][:, kc % 2, :],
                                                 st_ps[:], AF.Sigmoid)
                        else:
                            v = v_p.tile([128, DIM], BF16, name="v", tag="v")
                            nc.vector.tensor_add(v[:], v_ps[:], bvb[:])
                            v_sb.append(v)
                            s = st_p.tile([128, N], BF16, name="s", tag="st")
                            nc.scalar.activation(s[:], st_ps[:], AF.Sigmoid)
                            st_sb.append(s)
                    return v_sb, st_sb

                def oproj(state, last):
                    # wp is folded into wv on the host, so the score-value
                    # product lands directly in output-channel space.
                    v_sb, st_sb = state
                    for dm in range(C3):
                        dsl = slice(dm * 128, (dm + 1) * 128)
                        o_ps = psum_pool.tile([128, N], F32, name="o_ps", tag="ps")
                        if fp8:
                            for p in range(HEADS // 2):
                                for hlf in range(2):
                                    sl = slice(hlf * HALF, (hlf + 1) * HALF)
                                    MM(o_ps[:, sl], v_sb[p][:, :, dsl],
                                       st_sb[p][:, :, sl],
                                       start=(p == 0), stop=(p == HEADS // 2 - 1),
                                       perf_mode=DR)
                        else:
                            for kc in range(HEADS):
                                for hlf in range(2):
                                    sl = slice(hlf * HALF, (hlf + 1) * HALF)
                                    MM(o_ps[:, sl], v_sb[kc][:, dsl],
                                       st_sb[kc][:, sl],
                                       start=(kc == 0), stop=(kc == HEADS - 1))
                        nc.vector.tensor_add(Y[dm][:], o_ps[:], Y[dm][:])
                        if last and on_tile_done is not None:
                            on_tile_done(dm, Y[dm])

                state = qkvst(0)
                for h in range(1, HEADS):
                    nxt = qkvst(h)
                    oproj(state, False)
                    state = nxt
                oproj(state, True)
                return Y

            x0 = [xin[:, c, :] for c in range(C3)]

            with tc.tile_pool(name="ps", bufs=4, space="PSUM") as psp:
                with (
                    tc.tile_pool(name="wq", bufs=4) as wq_p,
                    tc.tile_pool(name="wv", bufs=3) as wv_p,
                    tc.tile_pool(name="st", bufs=20) as st_p,
                    tc.tile_pool(name="v", bufs=20) as v_p,
                    tc.tile_pool(name="qk", bufs=6) as qk_p,
                    tc.tile_pool(name="bvb", bufs=2) as bvb_p,
                ):
                    pools = (wq_p, wv_p, st_p, v_p, qk_p, bvb_p)

                    # PEG depthwise 3x3 conv, emitted per tile from inside
                    # MHA1's final o-projection. The center tap runs on ACT,
                    # the two vertical taps on GpSimd (second accumulator),
                    # the remaining six on the DVE; the merge writes the bf16
                    # tile LN1 consumes directly. Each tile's LN square is
                    # emitted one callback late so the ACT queue never blocks
                    # the next tile's center tap.
                    x2 = [None] * C3
                    sq1 = [None] * C3
                    acc2s = [None] * C3

                    def peg_tile(c, y):
                        acc = xp.tile([128, N], F32, name="peg_acc", tag="x")
                        nc.scalar.activation(
                            acc[:], y[:], AF.Identity,
                            bias=pegb_sb[:, c:c + 1], scale=pegw_sb[:, c, 4:5])
                        acc2 = xp.tile([128, N], F32, name="peg_acc2", tag="x")
                        acc2s[c] = acc2
                        a3d = acc[:].rearrange("p (h w) -> p h w", w=W)
                        b3d = acc2[:].rearrange("p (h w) -> p h w", w=W)
                        x3d = y[:].rearrange("p (h w) -> p h w", w=W)
                        # vertical taps (dy=-1 tap1, dy=+1 tap7) on GpSimd.
                        # Pool has no per-partition-scalar opcode, so tap7 is
                        # prescaled on ACT and the Pool ops are a broadcast
                        # multiply, a row copy, and an in-place add.
                        tp = xp.tile([128, N], F32, name="peg_tp", tag="x")
                        nc.scalar.activation(tp[:], y[:], AF.Identity,
                                             scale=pegw_sb[:, c, 7:8])
                        tp3d = tp[:].rearrange("p (h w) -> p h w", w=W)
                        nc.gpsimd.tensor_tensor(
                            b3d[:, 1:H, :], x3d[:, 0:H - 1, :],
                            pegw_sb[:, c, 1:2].broadcast_to([128, H - 1, W]),
                            op=OP.mult)
                        nc.gpsimd.tensor_copy(b3d[:, 0:1, :], tp3d[:, 1:2, :])
                        nc.gpsimd.tensor_tensor(
                            b3d[:, 1:H - 1, :], b3d[:, 1:H - 1, :],
                            tp3d[:, 2:H, :], op=OP.add)
                        for dy in (-1, 0, 1):
                            for dx in (-1, 1):
                                tap = 3 * (dy + 1) + (dx + 1)
                                oh = slice(max(0, -dy), H - max(0, dy))
                                ow = slice(max(0, -dx), W - max(0, dx))
                                ih = slice(max(0, dy), H + min(0, dy))
                                iw = slice(max(0, dx), W + min(0, dx))
                                nc.vector.scalar_tensor_tensor(
                                    a3d[:, oh, ow], x3d[:, ih, iw],
                                    pegw_sb[:, c, tap:tap + 1], a3d[:, oh, ow],
                                    op0=OP.mult, op1=OP.add)
                        accf = xb.tile([128, N], BF16, name="peg_out", tag="xb")
                        nc.vector.tensor_add(accf[:], acc[:], acc2[:])
                        x2[c] = accf
                        if c > 0:
                            sq = xb.tile([128, N], BF16, name="sq", tag="xb")
                            nc.scalar.square(sq[:], x2[c - 1][:])
                            sq1[c - 1] = sq

                    x1 = mha(1, x0, pools, psp, on_tile_done=peg_tile)
                    sq = xb.tile([128, N], BF16, name="sq", tag="xb")
                    nc.scalar.square(sq[:], x2[C3 - 1][:])
                    sq1[C3 - 1] = sq

                    warm_ps = psp.tile([1, 64], F32, name="warm", tag="ps")

                    def warm1(c):
                        # keep the PE's HAM clock gate warm across the last
                        # PEG tile: row 31 of acc2[c2] is written by GpSimd's
                        # first op, so this matmul fires mid-valley.
                        if c == C3 - 1:
                            src = acc2s[c][:, (H - 1) * W:N]
                            MM(warm_ps[:, 0:W], src[:, 0:1], src,
                               start=True, stop=True)

                    x3 = layer_norm(x2, "ln1", psp, BF16, xb,
                                    shadows=list(zip(x2, sq1)),
                                    pre_colsum=warm1)

                    # MHA2; its last o-projection also emits the a3 residual
                    # and the mlpln shadow copies per tile so the MLP's LN
                    # starts the moment the attention ends.
                    u_sb = [None] * C3
                    xscm = [None] * C3
                    sqm = [None] * C3

                    def mlp_prep(dm, y):
                        u = xp.tile([128, N], F32, name="u", tag="x")
                        nc.vector.tensor_scalar(
                            u[:], y[:], a3_sb[:, dm:dm + 1], b2_sb[:, dm:dm + 1],
                            op0=OP.mult, op1=OP.add)
                        u_sb[dm] = u
                        xsc = xb.tile([128, N], BF16, name="xs", tag="xb")
                        nc.scalar.copy(xsc[:], y[:])
                        xscm[dm] = xsc
                        sq = xb.tile([128, N], BF16, name="sq", tag="xb")
                        nc.scalar.square(sq[:], y[:])
                        sqm[dm] = sq

                    x4 = mha(2, x3, pools, psp, fp8=True, on_tile_done=mlp_prep)

                layer_norm(x4, "mlpln", psp,
                           out_aps=[hn_t[:, c, :] for c in range(C3)],
                           shadows=list(zip(xscm, sqm)))

            # ---- MLP: w1 -> gelu -> w2, fp8 DoubleRow, pipelined per
            # ht-pair. o2 accumulators live across the whole stream, so they
            # get their own PSUM scope (3x2 banks) next to the hd halves
            # (2x1 banks).
            with (
                tc.tile_pool(name="o2ps", bufs=3, space="PSUM") as o2p,
                tc.tile_pool(name="hdps", bufs=2, space="PSUM") as hdp,
            ):
                o2_ps = [o2p.tile([128, N], F32, name="o2_ps", tag="o2")
                         for _ in range(C3)]
                for ht in range(H6):
                    hsl = slice(ht * 128, (ht + 1) * 128)
                    for hlf in range(2):
                        sl = slice(hlf * HALF, (hlf + 1) * HALF)
                        hd = hdp.tile([128, HALF], F32, name="hd", tag="hd")
                        MM(hd[:], w1_sb[:, 0:2, hsl], hn_t[:, 0:2, sl],
                           start=True, stop=False, perf_mode=DR)
                        MM(hd[:], w1_sb[:, 2, hsl], hn_t[:, 2, sl],
                           start=False, stop=True)
                        nc.scalar.activation(hid_t[:, ht, sl], hd[:], AF.Gelu,
                                             bias=b1_sb[:, ht:ht + 1])
                    if ht % 2 == 1:
                        j = ht // 2
                        psl = slice(ht - 1, ht + 1)
                        for dm in range(C3):
                            dsl = slice(dm * 128, (dm + 1) * 128)
                            for hlf in range(2):
                                sl = slice(hlf * HALF, (hlf + 1) * HALF)
                                MM(o2_ps[dm][:, sl], w2_sb[:, psl, dsl],
                                   hid_t[:, psl, sl],
                                   start=(j == 0), stop=(j == H6 // 2 - 1),
                                   perf_mode=DR)
                x5 = []
                sh5 = []
                for dm in range(C3):
                    y = xp.tile([128, N], F32, name="x5t", tag="x")
                    nc.vector.tensor_add(y[:], o2_ps[dm][:], u_sb[dm][:])
                    x5.append(y)
                    xsc = xb.tile([128, N], BF16, name="xs", tag="xb")
                    nc.scalar.copy(xsc[:], y[:])
                    sq = xb.tile([128, N], BF16, name="sq", tag="xb")
                    nc.scalar.square(sq[:], y[:])
                    sh5.append((xsc, sq))

            with tc.tile_pool(name="ps2", bufs=4, space="PSUM") as psp2:
                yout = layer_norm(x5, "ln2", psp2, BF16, xb, shadows=sh5)
                for c in range(C3):
                    nc.sync.dma_start(out_ext[c], yout[c][:])

    nc.compile()
    return nc


def _prep_weights(inputs):
    """Host-side reshapes into SBUF-tile-friendly layouts."""
    g = {k: np.ascontiguousarray(np.asarray(v, dtype=np.float32))
         for k, v in inputs.items()}
    s = 1.0 / math.sqrt(DK)
    bf = ml_dtypes.bfloat16
    f8 = ml_dtypes.float8_e4m3fn
    m = {}
    for i in (1, 2):
        wq = g[f"wq{i}"] * s                      # fold score scale into Q
        m[f"wq{i}"] = wq.reshape(HEADS, C3, 128, DK).transpose(0, 2, 1, 3).astype(bf)
        m[f"wk{i}"] = g[f"wk{i}"].reshape(HEADS, C3, 128, DK).transpose(0, 2, 1, 3).astype(bf)
        wp = g[f"wp{i}"].reshape(HEADS, DIM, DIM)          # [h, 384, 384]
        wvp = np.einsum("hdf,hfe->hde", g[f"wv{i}"], wp)   # fold projector
        bvp = np.einsum("hf,hfe->he", g[f"bv{i}"], wp)
        m[f"wv{i}"] = wvp.reshape(HEADS, C3, 128, DIM).transpose(0, 2, 1, 3).astype(bf)
        m[f"bv{i}"] = bvp.astype(bf)              # [8, 384]
    m["mlp_w1"] = g["mlp_w1"].reshape(C3, 128, 768).transpose(1, 0, 2).astype(f8)
    m["mlp_w2"] = g["mlp_w2"].reshape(H6, 128, DIM).transpose(1, 0, 2).astype(f8)

    def col3(v):
        return np.asarray(v, np.float32).reshape(DIM).reshape(C3, 128).T

    cf = np.zeros((128, 96), np.float32)
    for j, k in enumerate(("ln1", "mlpln", "ln2")):
        cf[:, 3 * j:3 * j + 3] = col3(g[f"{k}_b"])
    cf[:, 9:12] = col3(g["a1"]); cf[:, 12:15] = col3(g["a2"])
    cf[:, 15:18] = col3(g["a3"])
    cf[:, 18:21] = col3(g["bp1"]); cf[:, 21:24] = col3(g["bp2"])
    cf[:, 24:51] = g["peg_w"].reshape(DIM, 9).reshape(C3, 128, 9).transpose(
        1, 0, 2).reshape(128, 27)
    cf[:, 51:54] = col3(g["peg_b"])
    cf[:, 54:60] = g["mlp_b1"].reshape(H6, 128).T
    cf[:, 60:63] = col3(g["mlp_b2"])
    cf[0:DK, 63:71] = (g["bq1"] * s).T
    cf[0:DK, 71:79] = g["bk1"].T
    cf[0:DK, 79:87] = (g["bq2"] * s).T
    cf[0:DK, 87:95] = g["bk2"].T
    m["constf"] = cf
    cg = np.concatenate([g[f"{k}_g"].reshape(DIM)
                         for k in ("ln1", "mlpln", "ln2")]).reshape(1, 3 * DIM)
    m["constg"] = cg.astype(bf)
    m = {k: np.ascontiguousarray(v) for k, v in m.items()}
    return m, g


_NC_CACHE = None


def kernel(**inputs) -> np.ndarray:
    global LAST_EXEC_TIME_NS, _NC_CACHE
    weights, g = _prep_weights(inputs)
    bf = ml_dtypes.bfloat16
    dec = g["decoder"].reshape(B, C3, 128, N).transpose(0, 2, 1, 3).astype(bf)

    if _NC_CACHE is None:
        _NC_CACHE = build_nc()
    nc = _NC_CACHE

    in_maps = []
    for b in range(B):
        im = {"x": np.ascontiguousarray(dec[b])}
        im.update(weights)
        in_maps.append(im)

    trace = bool(int(os.environ.get("KERNEL_TRACE", "0")))
    if trace:
        trace = _install_profile_hook()
    res = run_bass_kernel_spmd(nc, in_maps, core_ids=list(range(B)), trace=trace)
    LAST_EXEC_TIME_NS = res.exec_time_ns

    out = np.stack([np.asarray(res.results[b]["out"]).astype(np.float32)
                    for b in range(B)], axis=0)
    return np.ascontiguousarray(out.reshape(B, DIM, H, W))


def _install_profile_hook():
    """Register the axon NTFF profiling hook this image's antenv lacks."""
    import sys
    import types
    try:
        from concourse import bass_utils as _bu
        _bu.upload_artifacts = lambda tmpdir: tmpdir
        try:
            import antenv.axon_hooks  # noqa: F401
            return True
        except ImportError:
            pass
        import antenv
        mod = types.ModuleType("antenv.axon_hooks")
        state = {"hook": None}
        mod.set_axon_ntff_profile_hook = lambda h: state.__setitem__("hook", h)
        mod.get_axon_ntff_profile_hook = lambda: state["hook"]
        sys.modules["antenv.axon_hooks"] = mod
        antenv.axon_hooks = mod
        from trn_agent_boot.trn_boot import _ntff_profile_via_ctypes
        mod.set_axon_ntff_profile_hook(
            _ntff_profile_via_ctypes("/opt/axon/libaxon_pjrt.so"))
        return True
    except Exception:
        return False
